# revision 8
# baseline (speedup 1.0000x reference)
"""Multi-head attention (B=2, T=4096, D=768, H=12) as a Bass/Tile kernel
for 8 Trainium2 NeuronCores.

Sharding: cores 0-3 own batch 0, cores 4-7 own batch 1; each core owns 3
heads. Host folds all bias constants (b_o and the b_v @ W_o terms) into a
single per-batch row added after the cross-core partial-sum gather.

Per-core pipeline:
  A) x^T arrives bf16. Q^T/K^T projections run bf16 (W stationary, x^T
     moving); the PSUM->SBUF conversion (ACT, Identity+bias) adds
     b_q/b_k and quantizes straight to fp8 e4m3. V stays bf16 in
     per-key-chunk V_aug tiles [128, 3*65] whose per-head 65th column is
     1.0.
  B) scores^T[k, q] = K^T-chunk.T @ Q^T as an fp8 DoubleRow matmul (the
     second k-tile of the pair points at a zeroed column range, so the
     product is unchanged); all three heads of a key chunk land in one
     [128, 1536] PSUM tile and take ONE exp op. exp alternates between
     ACT (true exp, scale fused, bf16 out) and DVE (one-op Schraudolph:
     fp32 affine s*AS + BS rounds to an integer whose low half-word IS
     the bf16 bit pattern of ~exp(s/8); read back via stride-2 bf16
     view).
  C) attn[q, 65]_h accumulates exp-chunk.T @ V_aug over 32 key chunks
     (moving is the 65-wide V_aug; column 64 yields sumexp[q] per
     partition). A single DVE tensor_tensor with a stride-0-broadcast
     1/sumexp view normalizes [q, 195] to bf16; two XBAR DMA transposes
     produce the [a, q] stationaries for the W_o projection (N=768
     moving); the normalized aug columns == 1 land on zeroed W_o rows.
"""
import sys
import os
import numpy as np

try:
    import jax
    jax.config.update("jax_compilation_cache_dir", "/tmp/jax_cache_mha")
    jax.config.update("jax_persistent_cache_min_compile_time_secs", 1.0)
except Exception:
    pass

if "/opt/trn_rl_repo" not in sys.path:
    sys.path.insert(0, "/opt/trn_rl_repo")

N_CORES = 8
B, T, D, H, DK = 2, 4096, 768, 12, 64
HPC = 3           # heads per core
NKC = T // 128    # 32 key chunks
NQB = T // 512    # 8 query blocks
LAG = int(os.environ.get("K_LAG", "4"))
TAIL1_KC = int(os.environ.get("K_T1", "1"))
TAIL2_KC = int(os.environ.get("K_T2", "8"))
EBUF = int(os.environ.get("K_EBUF", "6"))
FP8 = int(os.environ.get("K_FP8", "1"))

# Per-kc exp engine: 'A' = ACT true exp, 'D' = DVE Schraudolph (61% A)
EXP_PAT = os.environ.get("K_PAT", "AADADAADADAADAADAD")

# Schraudolph: low half-word of fp32(s*AS + BS) is the bf16 bit pattern of
# exp(s*0.125)*(1+eps). AS = 0.125*128/ln2. BS scales by c = E[rho]/E[rho^2]
# (rho(f) = (1+f)/2^f), minimizing the RMS of eps: rms 1.8%, |eps| <= 4%.
_AS = 0.125 * 128.0 / np.log(2.0)
_F = np.linspace(0, 1, 200001)[:-1]
_RHO = (1 + _F) / np.exp2(_F)
_BS = 12582912.0 + 16256.0 + 128.0 * np.log2(_RHO.mean() / (_RHO ** 2).mean())

_cache = {}


def _build_nc():
    import concourse.bass as bass  # noqa: F401
    import concourse.mybir as mybir
    import concourse.tile as tile
    from concourse import bacc

    f32 = mybir.dt.float32
    bf16 = mybir.dt.bfloat16
    fp8 = mybir.dt.float8e4
    qk_dt = fp8 if FP8 else bf16
    AF = mybir.ActivationFunctionType
    ALU = mybir.AluOpType
    DR = mybir.MatmulPerfMode.DoubleRow

    nc = bacc.Bacc(None, target_bir_lowering=False)
    xbT = nc.dram_tensor("xbT", [D, T], bf16, kind="ExternalInput")
    wqk = nc.dram_tensor("wqk", [D, 384], bf16, kind="ExternalInput")
    wv = nc.dram_tensor("wv", [D, 192], bf16, kind="ExternalInput")
    wo1 = nc.dram_tensor("wo1", [128, D], bf16, kind="ExternalInput")
    wo2 = nc.dram_tensor("wo2", [67, D], bf16, kind="ExternalInput")
    bpack = nc.dram_tensor("bpack", [128, 3], f32, kind="ExternalInput")
    o = nc.dram_tensor("o", [T, D], f32, kind="ExternalOutput")

    QW = 2 * T if FP8 else T  # Q/K tile width (fp8 keeps a zeroed 2nd half)

    with tile.TileContext(nc) as tc:
        with tc.tile_pool(name="pers", bufs=1) as pers, \
             tc.tile_pool(name="expp", bufs=EBUF) as expp, \
             tc.tile_pool(name="attn", bufs=4) as attnp, \
             tc.tile_pool(name="accp", bufs=2, space="PSUM") as accp, \
             tc.tile_pool(name="shp", bufs=2, space="PSUM") as shp:

            # ---------------- persistent SBUF ----------------
            wqk_t = pers.tile([128, 6 * 384], bf16, tag="wqk")
            nc.sync.dma_start(
                out=wqk_t.rearrange("p (a c) -> p a c", a=6),
                in_=wqk[:, :].rearrange("(a p) c -> p a c", p=128))
            wv_t = pers.tile([128, 6 * 192], bf16, tag="wv")
            nc.sync.dma_start(
                out=wv_t.rearrange("p (a c) -> p a c", a=6),
                in_=wv[:, :].rearrange("(a p) c -> p a c", p=128))
            wo1_t = pers.tile([128, D], bf16, tag="wo1")
            nc.sync.dma_start(out=wo1_t, in_=wo1[:, :])
            wo2_t = pers.tile([67, D], bf16, tag="wo2")
            nc.sync.dma_start(out=wo2_t, in_=wo2[:, :])
            bias_t = pers.tile([128, 3], f32, tag="bias")
            nc.sync.dma_start(out=bias_t, in_=bpack[:, :])

            xt = [pers.tile([128, T], bf16, tag=f"xt{dc}", name=f"xt{dc}")
                  for dc in range(6)]
            for dc in range(6):
                nc.sync.dma_start(out=xt[dc],
                                  in_=xbT[dc * 128:(dc + 1) * 128, :])

            # Q/K tiles (fp8 or bf16); fp8 keeps cols T..2T zeroed for the
            # DoubleRow dummy second k-tile.
            qA = pers.tile([128, QW], qk_dt, tag="qA")
            kA = pers.tile([128, QW], qk_dt, tag="kA")
            qB = pers.tile([64, QW], qk_dt, tag="qB")
            k2s = pers.tile([128, QW], qk_dt, tag="k2s")  # rows 64:128 used
            kB = pers.tile([64, QW], qk_dt, tag="kB")
            if FP8:
                for t_ in (qA, kA, qB, kB):
                    nc.gpsimd.memset(t_[:, T:2 * T], 0.0)

            # V_aug: per key chunk [128, 3*65] bf16, col 65h+64 = 1.0
            vaug = pers.tile([128, NKC * 195], bf16, tag="vaug")
            vaug4 = vaug.rearrange("p (k h c) -> p k h c", k=NKC, h=3)
            nc.gpsimd.memset(vaug4[:, :, :, 64], 1.0)

            exp_tiles = {}   # (b, kc) -> (kind, tile)
            tailst = {}      # (b, q4) -> (aT1, aT2, rc)

            def qk_ap(t_, rows, cs):
                """[rows, 2, len(cs)] AP: k-tile pair (data, zeros)."""
                return t_.rearrange("p (j c) -> p j c", j=2)[rows, :, cs]

            # ---------------- emit helpers ----------------
            def emit_scores_exp(b, kc):
                qs = slice(b * 512, (b + 1) * 512)
                ks = slice(kc * 128, (kc + 1) * 128)
                sc = shp.tile([128, 1536], f32, tag="ps", name="sc")
                if FP8:
                    nc.tensor.matmul(
                        sc[:, 0:512], qk_ap(kA, slice(0, 64), ks),
                        qk_ap(qA, slice(0, 64), qs), perf_mode=DR,
                        start=True, stop=True, skip_group_check=True)
                    nc.tensor.matmul(
                        sc[:, 512:1024], qk_ap(kA, slice(64, 128), ks),
                        qk_ap(qA, slice(64, 128), qs), perf_mode=DR,
                        start=True, stop=True, tile_position=(64, 0),
                        skip_group_check=True)
                    nc.tensor.matmul(
                        sc[:, 1024:1536], qk_ap(kB, slice(0, 64), ks),
                        qk_ap(qB, slice(0, 64), qs), perf_mode=DR,
                        start=True, stop=True, skip_group_check=True)
                else:
                    nc.tensor.matmul(sc[:, 0:512], kA[0:64, ks], qA[0:64, qs],
                                     start=True, stop=True,
                                     skip_group_check=True)
                    nc.tensor.matmul(sc[:, 512:1024], kA[64:128, ks],
                                     qA[64:128, qs], start=True, stop=True,
                                     tile_position=(64, 0),
                                     skip_group_check=True)
                    nc.tensor.matmul(sc[:, 1024:1536], kB[:, ks], qB[:, qs],
                                     start=True, stop=True,
                                     skip_group_check=True)
                eng = EXP_PAT[kc % len(EXP_PAT)]
                if eng == "A":
                    e = expp.tile([128, 1536], bf16, tag="ea", name="ea")
                    nc.scalar.activation(e, sc, AF.Exp, scale=0.125)
                else:
                    e = expp.tile([128, 1536], f32, tag="eb", name="eb")
                    nc.vector.tensor_scalar(e, sc, float(_AS), float(_BS),
                                            ALU.mult, ALU.add)
                exp_tiles[(b, kc)] = (eng, e)

            def emit_attnv(b, kc, acc01, acc23):
                eng, e = exp_tiles.pop((b, kc))
                if eng == "A":
                    full = e
                else:
                    full = e.bitcast(bf16).rearrange(
                        "p (c x) -> p c x", x=2)[:, :, 0]
                for h in range(HPC):
                    for q4 in range(4):
                        stat = full[:, h * 512 + q4 * 128:
                                    h * 512 + q4 * 128 + 128]
                        acc = acc01 if q4 < 2 else acc23
                        off = (q4 & 1) * 256 + h * 65
                        nc.tensor.matmul(
                            acc[:, off:off + 65], stat,
                            vaug[:, kc * 195 + h * 65:kc * 195 + h * 65 + 65],
                            start=(kc == 0 and h == 0 and (q4 & 1) == 0),
                            stop=(kc == NKC - 1 and h == HPC - 1
                                  and (q4 & 1) == 1),
                            skip_group_check=True)

            def emit_tail1(b, acc01, acc23):
                """recip + stride-0-broadcast normalize (bf16) + XBAR."""
                for q4 in range(4):
                    acc = acc01 if q4 < 2 else acc23
                    off = (q4 & 1) * 256
                    rc = attnp.tile([128, 4], f32, tag="rc", name="rc")
                    se = acc[:, off:off + 195].rearrange(
                        "p (c x) -> p c x", x=65)[:, :, 64]
                    nc.vector.reciprocal(rc[:, 0:3], se)
                    an = attnp.tile([128, 256], bf16, tag="an", name="an")
                    rcb = rc[:, 0:3].unsqueeze(2).broadcast_to([128, 3, 65])
                    nc.vector.tensor_tensor(
                        an[:, 0:195].rearrange("p (h c) -> p h c", h=3),
                        acc[:, off:off + 195].rearrange(
                            "p (h c) -> p h c", h=3),
                        rcb, ALU.mult)
                    nc.gpsimd.memset(an[:, 195:256], 0.0)
                    aT1 = attnp.tile([128, 128], bf16, tag="aT1", name="aT1")
                    aT2 = attnp.tile([128, 128], bf16, tag="aT2", name="aT2")
                    nc.sync.dma_start_transpose(aT1, an[:, 0:128])
                    nc.sync.dma_start_transpose(aT2, an[:, 128:256])
                    tailst[(b, q4)] = (aT1, aT2)

            def emit_tail2(b):
                """O-projection + store for block b (after tail1)."""
                for q4 in range(4):
                    aT1, aT2 = tailst.pop((b, q4))
                    op = shp.tile([128, 1536], f32, tag="ps", name="op")
                    for half in range(2):
                        oc = slice(half * 512, half * 512 + 384)
                        hc = slice(half * 384, half * 384 + 384)
                        nc.tensor.matmul(op[:, oc], aT1, wo1_t[:, hc],
                                         start=True, stop=False,
                                         skip_group_check=True)
                        nc.tensor.matmul(op[:, oc], aT2[0:67, :],
                                         wo2_t[:, hc], start=False, stop=True,
                                         skip_group_check=True)
                    ot = attnp.tile([128, D], f32, tag="ot", name="ot")
                    nc.vector.tensor_copy(ot[:, 0:384], op[:, 0:384])
                    nc.vector.tensor_copy(ot[:, 384:768], op[:, 512:896])
                    r0 = b * 512 + q4 * 128
                    nc.sync.dma_start(out=o[r0:r0 + 128, :], in_=ot)

            # ---------------- phase A (+ block-0 interleave) ----------------
            b0_acc01 = accp.tile([128, 512], f32, tag="acc", name="acc0")
            b0_acc23 = accp.tile([128, 512], f32, tag="acc", name="acc1")

            for t in range(8):
                tcols = slice(t * 512, (t + 1) * 512)
                for g in range(3):
                    pj = shp.tile([128, 1536], f32, tag="ps", name="pj")
                    for dc in range(6):
                        nc.tensor.matmul(
                            pj[:, 0:512], wqk_t[:, dc * 384 + g * 128:
                                                dc * 384 + (g + 1) * 128],
                            xt[dc][:, tcols], start=(dc == 0),
                            stop=(dc == 5), skip_group_check=True)
                    # PSUM->SBUF: add bias, quantize to fp8/bf16 (on ACT)
                    if g == 0:
                        nc.scalar.activation(qA[:, tcols], pj[:, 0:512],
                                             AF.Identity,
                                             bias=bias_t[:, 0:1])
                    elif g == 1:
                        nc.scalar.activation(kA[:, tcols], pj[:, 0:512],
                                             AF.Identity,
                                             bias=bias_t[:, 1:2])
                    else:
                        nc.scalar.activation(qB[:, tcols], pj[0:64, 0:512],
                                             AF.Identity,
                                             bias=bias_t[0:64, 2:3])
                        nc.scalar.activation(k2s[64:128, tcols],
                                             pj[64:128, 0:512],
                                             AF.Identity,
                                             bias=bias_t[64:128, 2:3])
                if t % 2 == 1:
                    sh = slice((t - 1) * 512, (t + 1) * 512)
                    nc.sync.dma_start(out=kB[:, sh], in_=k2s[64:128, sh])
                for i in range(4):
                    kc = t * 4 + i
                    vp = shp.tile([128, 1536], f32, tag="ps", name="vp")
                    for dc in range(6):
                        nc.tensor.matmul(
                            vp[:, 0:192],
                            xt[dc][:, kc * 128:(kc + 1) * 128],
                            wv_t[:, dc * 192:(dc + 1) * 192],
                            start=(dc == 0), stop=(dc == 5),
                            skip_group_check=True)
                    nc.vector.tensor_copy(
                        vaug4[:, kc, :, 0:64],
                        vp[:, 0:192].rearrange("p (h c) -> p h c", h=3))
                if t >= 1:
                    for kc in range(4 * (t - 1), 4 * t):
                        emit_scores_exp(0, kc)
                        if kc >= LAG:
                            emit_attnv(0, kc - LAG, b0_acc01, b0_acc23)

            # ---------------- blocks ----------------
            carry = None
            cur = (0, b0_acc01, b0_acc23)
            for b in range(NQB):
                if b == 0:
                    for kc in range(28, 32):
                        emit_scores_exp(0, kc)
                        emit_attnv(0, kc - LAG, cur[1], cur[2])
                    for kc in range(NKC - LAG, NKC):
                        emit_attnv(0, kc, cur[1], cur[2])
                else:
                    acc01 = accp.tile([128, 512], f32, tag="acc", name="acc0")
                    acc23 = accp.tile([128, 512], f32, tag="acc", name="acc1")
                    cur = (b, acc01, acc23)
                    for kc in range(NKC):
                        emit_scores_exp(b, kc)
                        if kc >= LAG:
                            emit_attnv(b, kc - LAG, acc01, acc23)
                        if kc == TAIL1_KC and carry is not None:
                            emit_tail1(carry[0], carry[1], carry[2])
                        if kc == TAIL2_KC and carry is not None:
                            emit_tail2(carry[0])
                    for kc in range(NKC - LAG, NKC):
                        emit_attnv(b, kc, acc01, acc23)
                carry = cur
            emit_tail1(carry[0], carry[1], carry[2])
            emit_tail2(carry[0])

    nc.finalize()
    return nc


def _get_nc():
    if "nc" not in _cache:
        _cache["nc"] = _build_nc()
    return _cache["nc"]


def _make_in_maps(x, W_q, b_q, W_k, b_k, W_v, b_v, W_o, b_o):
    import ml_dtypes
    bf = ml_dtypes.bfloat16
    in_maps = []
    for c in range(N_CORES):
        b = c // 4
        h0 = (c % 4) * HPC
        c0 = h0 * DK

        g0 = W_q[:, c0:c0 + 128]
        g1 = W_k[:, c0:c0 + 128]
        g2 = np.concatenate([W_q[:, c0 + 128:c0 + 192],
                             W_k[:, c0 + 128:c0 + 192]], axis=1)
        wqk_m = np.concatenate([g0, g1, g2], axis=1)

        bpack = np.zeros((128, 3), np.float32)
        bpack[:, 0] = b_q[c0:c0 + 128]
        bpack[:, 1] = b_k[c0:c0 + 128]
        bpack[0:64, 2] = b_q[c0 + 128:c0 + 192]
        bpack[64:128, 2] = b_k[c0 + 128:c0 + 192]

        # woaug [195, 768]: per head rows 0..63 = W_o rows; row 64 = 0
        woaug = np.zeros((195, D), np.float32)
        for j in range(HPC):
            woaug[j * 65:j * 65 + 64, :] = \
                W_o[c0 + j * DK:c0 + (j + 1) * DK, :]

        in_maps.append({
            "xbT": np.ascontiguousarray(x[b].T).astype(bf),
            "wqk": np.ascontiguousarray(wqk_m).astype(bf),
            "wv": np.ascontiguousarray(W_v[:, c0:c0 + 192]).astype(bf),
            "wo1": np.ascontiguousarray(woaug[0:128, :]).astype(bf),
            "wo2": np.ascontiguousarray(woaug[128:195, :]).astype(bf),
            "bpack": bpack,
        })
    return in_maps


def kernel(**inputs):
    from concourse.bass_utils import run_bass_kernel_spmd

    args = {k: np.asarray(v, dtype=np.float32) for k, v in inputs.items()}
    in_maps = _make_in_maps(
        args["x"], args["W_q"], args["b_q"], args["W_k"], args["b_k"],
        args["W_v"], args["b_v"], args["W_o"], args["b_o"])

    nc = _get_nc()
    trace = bool(int(os.environ.get("KBENCH_TRACE", "0")))
    res = run_bass_kernel_spmd(nc, in_maps, core_ids=list(range(N_CORES)),
                               trace=trace)
    _cache["last_result"] = res

    out = np.zeros((B, T, D), np.float32)
    for c in range(N_CORES):
        out[c // 4] += res.results[c]["o"]
    # bias constants folded on host: b_o plus every head's b_v @ W_o
    bias_row = args["b_o"] + args["b_v"] @ args["W_o"]
    out += bias_row[None, None, :]
    return out
